# revision 1
# baseline (speedup 1.0000x reference)
"""Trainium2 Bass kernel: 2-layer GRU (H=128) over 28 timesteps + Linear head.

Reference computation (PyTorch GRUCell semantics, gates r,z,n):
    for t in 28 rows of each 28x28 image:
        h1 = relu(gru1(x_t, h1));  h2 = relu(gru2(h1, h2))
    out = h2 @ w_out.T + b_out

Sharding: pure data parallel, batch 32768 -> 8 cores x 4096.
On-chip layout: transposed [hidden=partition, batch=free]; batch tiled 8x512.
Matmuls in bf16 (fp32 PSUM accumulate); gate elementwise in bf16.
Bias folding:
  - L1: x augmented with ones row; w1aug row 28 carries b_ih1(+b_hh1 for r,z).
  - L2: r/z biases via ScalarE activation bias; b_ih2n via tanh bias.
  - b_hh*n folded into the (ghn + b) * r fused scalar_tensor_tensor.
"""

import json
import os
from contextlib import ExitStack

import ml_dtypes
import numpy as np

import concourse.bass as bass
import concourse.tile as tile
from concourse import mybir
from concourse.bass_utils import run_bass_kernel_spmd

HID = 128
T = 28
C = 28
KAUG = C + 1
NCORES = 8
N_TOTAL = 32768
B_CORE = N_TOTAL // NCORES  # 4096
BF = 512                    # batch tile (matmul free dim / psum bank)
NSUB = B_CORE // BF         # 8
NOUT = 10

F32 = mybir.dt.float32
BF16 = mybir.dt.bfloat16
AF = mybir.ActivationFunctionType
ALU = mybir.AluOpType

# engine assignment knobs (tuned via profiling)
T3_ON_GPSIMD = False
RELU_ON_GPSIMD = False

# stash of the last run's perf results for test harness inspection
LAST_RESULT = None


def _split_multi_waits(bir_bytes: bytes) -> bytes:
    """This walrus build rejects instructions carrying >1 sync wait
    ("Too many sync wait commands"). Split extras into standalone
    single-wait EventSemaphore instructions on the same engine, placed
    immediately before -- semantically identical blocking."""
    d = json.loads(bir_bytes)
    ctr = 0
    for fn in d["functions"]:
        for bb in fn["blocks"]:
            out = []
            for inst in bb["instructions"]:
                si = inst.get("sync_info")
                waits = (si or {}).get("on_wait") or []
                if len(waits) > 1:
                    for w in waits[:-1]:
                        ctr += 1
                        out.append({
                            "debug": inst.get("debug", 0),
                            "engine": inst.get("engine"),
                            "ins": [],
                            "outs": [],
                            "name": f"xw-{ctr}",
                            "opcode": "EventSemaphore",
                            "sync_info": {"on_update": [], "on_wait": [w]},
                        })
                    si["on_wait"] = [waits[-1]]
                out.append(inst)
            bb["instructions"] = out
    return json.dumps(d).encode()


def _build_bass() -> bass.Bass:
    nc = bass.Bass()

    x = nc.dram_tensor("x", [(T + 2) // 3, 128, B_CORE], BF16, kind="ExternalInput")
    w1aug_d = nc.dram_tensor("w1aug", [128, 3 * HID], BF16, kind="ExternalInput")
    whh1_d = nc.dram_tensor("whh1T", [HID, 3 * HID], BF16, kind="ExternalInput")
    wih2_d = nc.dram_tensor("wih2T", [HID, 3 * HID], BF16, kind="ExternalInput")
    whh2_d = nc.dram_tensor("whh2T", [HID, 3 * HID], BF16, kind="ExternalInput")
    wout_d = nc.dram_tensor("woutT", [HID, NOUT], BF16, kind="ExternalInput")
    # bias columns: 0=b2r, 1=b2z, 2=b_hh1n, 3=b_hh2n, 4=b_ih2n
    bias_d = nc.dram_tensor("biases", [HID, 5], F32, kind="ExternalInput")
    bn_d = nc.dram_tensor("bnrow", [1, 2 * HID], BF16, kind="ExternalInput")
    bout_d = nc.dram_tensor("bout", [NOUT, BF], F32, kind="ExternalInput")
    out_d = nc.dram_tensor("out", [NOUT, B_CORE], F32, kind="ExternalOutput")

    with ExitStack() as ctx:
        tc = ctx.enter_context(tile.TileContext(nc))

        consts = ctx.enter_context(tc.tile_pool(name="consts", bufs=1))
        prz = ctx.enter_context(tc.tile_pool(name="prz", bufs=2, space="PSUM"))
        pgi = ctx.enter_context(tc.tile_pool(name="pgi", bufs=2, space="PSUM"))
        pgh = ctx.enter_context(tc.tile_pool(name="pgh", bufs=2, space="PSUM"))
        spool = ctx.enter_context(tc.tile_pool(name="sp", bufs=5))
        hpool = ctx.enter_context(tc.tile_pool(name="hp", bufs=3))
        opool = ctx.enter_context(tc.tile_pool(name="op", bufs=1))

        w1 = consts.tile([128, 3 * HID], BF16)
        nc.sync.dma_start(out=w1, in_=w1aug_d[:, :])
        wh1 = consts.tile([HID, 3 * HID], BF16)
        nc.sync.dma_start(out=wh1, in_=whh1_d[:, :])
        wi2 = consts.tile([HID, 3 * HID], BF16)
        nc.sync.dma_start(out=wi2, in_=wih2_d[:, :])
        wh2 = consts.tile([HID, 3 * HID], BF16)
        nc.sync.dma_start(out=wh2, in_=whh2_d[:, :])
        wo = consts.tile([HID, NOUT], BF16)
        nc.sync.dma_start(out=wo, in_=wout_d[:, :])
        bs = consts.tile([HID, 5], F32)
        nc.sync.dma_start(out=bs, in_=bias_d[:, :])
        bo = consts.tile([NOUT, BF], F32)
        nc.sync.dma_start(out=bo, in_=bout_d[:, :])
        bn = consts.tile([1, 2 * HID], BF16)
        nc.sync.dma_start(out=bn, in_=bn_d[:, :])
        ones_bf = consts.tile([1, BF], BF16)
        nc.vector.memset(ones_bf, 1.0)
        zeros_bf = consts.tile([HID, BF], BF16)
        nc.vector.memset(zeros_bf, 0.0)

        xg = []
        for g in range((T + 2) // 3):
            xt_ = consts.tile([128, B_CORE], BF16, tag=f"xg_{g}", name=f"xg_{g}")
            nc.sync.dma_start(out=xt_, in_=x[g, :, :])
            xg.append(xt_)

        h1 = {}
        h2 = {}
        for s in range(NSUB):
            h1[s] = hpool.tile([HID, BF], BF16, tag=f"h1_{s}", name=f"h1i_{s}")
            nc.vector.memset(h1[s], 0.0)
            h2[s] = hpool.tile([HID, BF], BF16, tag=f"h2_{s}", name=f"h2i_{s}")
            nc.vector.memset(h2[s], 0.0)

        def gru_cell(xa, kin, h_prev, w_ih, w_hh, sig_bias_r, sig_bias_z,
                     bhhn, tanh_bias, out_tag, wbase=0):
            """One GRU cell step + relu for one [*,BF] batch tile.

            xa: input-side rhs [kin, BF]; h_prev: [HID, BF] bf16.
            w_ih: lhsT [kin, 3*HID]; w_hh: lhsT [HID, 3*HID].
            sig_bias_*: None (pre-folded in psum) or [HID,1] AP for ACT bias.
            bhhn: [HID,1] AP, n-gate hidden bias (applied pre r-multiply).
            tanh_bias: 0.0 or [HID,1] AP (input-side n bias).
            """
            rz = prz.tile([HID, 2 * BF], F32, tag="rz")
            nc.tensor.matmul(rz[:, 0:BF], w_ih[wbase:wbase + kin, 0:HID], xa,
                             start=True, stop=False)
            nc.tensor.matmul(rz[:, 0:BF], w_hh[:, 0:HID], h_prev,
                             start=False, stop=True)
            nc.tensor.matmul(rz[:, BF:2 * BF], w_ih[wbase:wbase + kin, HID:2 * HID], xa,
                             start=True, stop=False)
            nc.tensor.matmul(rz[:, BF:2 * BF], w_hh[:, HID:2 * HID], h_prev,
                             start=False, stop=True)
            gi = pgi.tile([HID, BF], F32, tag="gi")
            nc.tensor.matmul(gi, w_ih[wbase:wbase + kin, 2 * HID:3 * HID], xa,
                             start=True, stop=True)
            gh = pgh.tile([HID, BF], F32, tag="gh")
            nc.tensor.matmul(gh, w_hh[:, 2 * HID:3 * HID], h_prev,
                             start=True, stop=True)

            rzs = spool.tile([HID, 2 * BF], BF16, tag="rzs")
            if sig_bias_r is None:
                nc.scalar.activation(rzs, rz, AF.Sigmoid)
            else:
                nc.scalar.activation(rzs[:, 0:BF], rz[:, 0:BF], AF.Sigmoid,
                                     bias=sig_bias_r)
                nc.scalar.activation(rzs[:, BF:2 * BF], rz[:, BF:2 * BF], AF.Sigmoid,
                                     bias=sig_bias_z)

            # t1 = (ghn + b_hhn) * r   (fused scalar_tensor_tensor)
            t1 = spool.tile([HID, BF], BF16, tag="t1")
            nc.vector.scalar_tensor_tensor(t1, gh, bhhn, rzs[:, 0:BF],
                                           op0=ALU.add, op1=ALU.mult)
            # t2 = t1 + gin
            t2 = spool.tile([HID, BF], BF16, tag="t2")
            nc.vector.tensor_tensor(t2, t1, gi, op=ALU.add)
            # n = tanh(t2 + b_ihn)
            nsb = spool.tile([HID, BF], BF16, tag="nsb")
            nc.scalar.activation(nsb, t2, AF.Tanh, bias=tanh_bias)
            # h' = n + z * (h - n), then relu
            t3 = spool.tile([HID, BF], BF16, tag="t3")
            if T3_ON_GPSIMD:
                nc.gpsimd.tensor_tensor(t3, h_prev, nsb, op=ALU.subtract)
            else:
                nc.vector.tensor_tensor(t3, h_prev, nsb, op=ALU.subtract)
            t4 = spool.tile([HID, BF], BF16, tag="t4")
            nc.vector.tensor_tensor(t4, rzs[:, BF:2 * BF], t3, op=ALU.mult)
            hp = spool.tile([HID, BF], BF16, tag="hpre")
            nc.vector.tensor_tensor(hp, t4, nsb, op=ALU.add)
            hn = hpool.tile([HID, BF], BF16, tag=out_tag, name=out_tag + "_n")
            nc.vector.tensor_scalar_max(hn, hp, 0.0)
            return hn

        for t in range(T):
            g, j = divmod(t, 3)
            for s in range(NSUB):
                xa = xg[g][32 * j:32 * j + KAUG, s * BF:(s + 1) * BF]
                h1[s] = gru_cell(xa, KAUG, h1[s], w1, wh1,
                                 None, None, bs[:, 2:3], 0.0, f"h1_{s}",
                                 wbase=32 * j)
                h2[s] = gru_cell(h1[s], HID, h2[s], wi2, wh2,
                                 bs[:, 0:1], bs[:, 1:2], bs[:, 3:4], bs[:, 4:5],
                                 f"h2_{s}")

        ob = opool.tile([NOUT, B_CORE], F32, tag="ob")
        for s in range(NSUB):
            po = pgi.tile([NOUT, BF], F32, tag="gi")
            nc.tensor.matmul(po, wo, h2[s], start=True, stop=True)
            nc.vector.tensor_tensor(ob[:, s * BF:(s + 1) * BF], po, bo, op=ALU.add)
        nc.scalar.dma_start(out=out_d[:, :], in_=ob)

    return nc


def _prep_inputs(x, w_ih1, w_hh1, b_ih1, b_hh1, w_ih2, w_hh2, b_ih2, b_hh2,
                 w_out, b_out):
    """Host-side reshape/transpose/cast + per-core sharding."""
    n = N_TOTAL
    xs = np.asarray(x, np.float32).reshape(n, T, C)       # channel dim is 1
    xt = np.transpose(xs, (1, 2, 0))                      # [T, C, n]
    xg = np.zeros(((T + 2) // 3, 128, n), np.float32)
    for t in range(T):
        g, j = divmod(t, 3)
        xg[g, 32 * j:32 * j + C, :] = xt[t]
        xg[g, 32 * j + C, :] = 1.0
    xg16 = xg.astype(ml_dtypes.bfloat16)

    w_ih1 = np.asarray(w_ih1, np.float32)
    w_hh1 = np.asarray(w_hh1, np.float32)
    b_ih1 = np.asarray(b_ih1, np.float32)
    b_hh1 = np.asarray(b_hh1, np.float32)
    w_ih2 = np.asarray(w_ih2, np.float32)
    w_hh2 = np.asarray(w_hh2, np.float32)
    b_ih2 = np.asarray(b_ih2, np.float32)
    b_hh2 = np.asarray(b_hh2, np.float32)
    w_out = np.asarray(w_out, np.float32)
    b_out = np.asarray(b_out, np.float32)

    H = HID
    w1aug = np.zeros((128, 3 * H), np.float32)
    bias_row = np.concatenate([
        b_ih1[0:H] + b_hh1[0:H],          # r: both biases
        b_ih1[H:2 * H] + b_hh1[H:2 * H],  # z: both biases
        b_ih1[2 * H:3 * H],               # n: input-side bias only
    ])
    for j in range(4):
        w1aug[32 * j:32 * j + C, :] = w_ih1.T
        w1aug[32 * j + C, :] = bias_row

    biases = np.stack([
        b_ih2[0:H] + b_hh2[0:H],
        b_ih2[H:2 * H] + b_hh2[H:2 * H],
        b_hh1[2 * H:3 * H],
        b_hh2[2 * H:3 * H],
        b_ih2[2 * H:3 * H],
    ], axis=1).astype(np.float32)         # [H, 5]

    common = {
        "w1aug": np.ascontiguousarray(w1aug.astype(ml_dtypes.bfloat16)),
        "whh1T": np.ascontiguousarray(w_hh1.T.astype(ml_dtypes.bfloat16)),
        "wih2T": np.ascontiguousarray(w_ih2.T.astype(ml_dtypes.bfloat16)),
        "whh2T": np.ascontiguousarray(w_hh2.T.astype(ml_dtypes.bfloat16)),
        "woutT": np.ascontiguousarray(w_out.T.astype(ml_dtypes.bfloat16)),
        "biases": np.ascontiguousarray(biases),
        "bout": np.ascontiguousarray(
            np.broadcast_to(b_out.reshape(NOUT, 1), (NOUT, BF)).astype(np.float32)),
        "bnrow": np.ascontiguousarray(np.concatenate([
            b_hh1[2 * H:3 * H], b_hh2[2 * H:3 * H]
        ]).reshape(1, 2 * H).astype(ml_dtypes.bfloat16)),
    }
    in_maps = []
    for c in range(NCORES):
        m = dict(common)
        m["x"] = np.ascontiguousarray(xg16[:, :, c * B_CORE:(c + 1) * B_CORE])
        in_maps.append(m)
    return in_maps


def kernel(**inputs):
    global LAST_RESULT
    nc = _build_bass()
    edited = _split_multi_waits(nc.to_json_bytes())
    nc.to_json_bytes = lambda: edited
    in_maps = _prep_inputs(**inputs)
    trace = bool(int(os.environ.get("BASS_TRACE", "0")))
    res = run_bass_kernel_spmd(nc, in_maps, core_ids=list(range(NCORES)),
                               trace=trace)
    LAST_RESULT = res
    outs = [r["out"] for r in res.results]          # each [NOUT, B_CORE] f32
    full = np.concatenate(outs, axis=1)             # [NOUT, N_TOTAL]
    return np.ascontiguousarray(full.T).astype(np.float32)

